# revision 21
# baseline (speedup 1.0000x reference)
"""Trainium2 Bass kernel for nn_BiT_Phoneme (dense transformer).

Data-parallel: 16 batch elems / 8 cores = 2 per core; each core runs the
full network on its 2 sequences (1024 "tokens"). Activations are kept
feature-major ([dim on partitions, tokens on free]) so matmuls chain
without transposes. The gaussian time-smoothing + patchify are a banded
matmul with a host-precomputed band matrix.

v2 redesign (vs baseline at 4.30 ms):
- LayerNorm gains/biases are folded into the *following* weight matrix on
  the host (W' = g*W, bias' += b@W), so the on-device LN apply is just
  (x - mu)*rstd -> bf16: 2 DVE ops/tile instead of 3, and the affine
  params never touch the device.
- rstd = reciprocal_approx_fast(sqrt(var+eps)) instead of the exact
  6 us/call nc.vector.reciprocal.
- mu/rstd partition-broadcasts run on GpSimd (partition_broadcast), not
  the PE, so LN never blocks the in-order PE queue.
- LNs are computed per 512-token half, and emission is dataflow-ordered:
  each half's stats+apply are emitted immediately after the phase that
  produces that half (wo b0 -> lnf h0 -> wo b1 -> lnf h1 -> ffn th0 ->
  lna' h0 -> ffn th1 -> lna' h1 -> next layer). The PE stays busy with
  the other half's matmuls while DVE/Scalar/GpSimd do the LN.
- Softmax denominators come free from a 65th all-ones column appended to
  each head's V block (probs@V matmul row 64 = sum of probs), removing
  768 M=1 PE matmuls.
- Score matmuls (K=64) for head pairs are emitted adjacently with
  explicit tile_position (0,0)/(64,0) so the two 64-row matmuls run
  concurrently in the PE array.
- q/k/v/wo/ffn drains moved to the Scalar engine (activation with
  per-partition bias) to balance DVE load.
Matmul dtypes: weights/activations bf16 (x residual stream fp32r).
"""

import numpy as np

import concourse.bass as bass
import concourse.mybir as mybir
import concourse.tile as tile
from concourse import bacc
from concourse.bass_utils import run_bass_kernel_spmd

B, T, F = 16, 2048, 256
PH = 4
PATCH = 1024
DIM = 1024
DEPTH = 6
HEADS, DHEAD = 16, 64
INNER = 1024
MLP = 4096
NCLS = 41
MAXREL = 200
KSIZE, SIGMA = 20, 2.0
EPS = 1e-5
SEQ = T // PH              # 512
NCORES = 8
BPC = B // NCORES          # 2
TOK = BPC * SEQ            # 1024
P = 128

DT_R = mybir.dt.float32r
DT_F = mybir.dt.float32
DT_H = mybir.dt.bfloat16
FX = mybir.ActivationFunctionType
OP = mybir.AluOpType

DTILES = DIM // P          # 8
KTILES = DIM // P          # 8
MTILES = MLP // P          # 32
SEQT = SEQ // P            # 4


def build_nc(p2_trivial, v_bias_zero, qk_bias_zero=True, dbg=False):
    nc = bacc.Bacc(None, target_bir_lowering=False)

    par = {}
    def dp(name, shape, dtype, is_out=False):
        par[name] = nc.declare_dram_parameter(name, list(shape), dtype, isOutput=is_out)
        return par[name]

    dp("xin", (BPC, T, F), DT_H)
    dp("band", (T // P, 3, P, P), DT_H)
    dp("etab", (DEPTH, SEQT, P, SEQ), DT_H)
    dp("wpe", (DTILES, P, KTILES, P), DT_H)
    dp("wqk_t", (DEPTH, 16, P, KTILES, P), DT_H)
    dp("wv_t", (DEPTH, 2, KTILES, P, 512), DT_H)
    dp("wot", (DEPTH, DTILES, P, KTILES, P), DT_H)
    dp("w1t", (DEPTH, MTILES, P, KTILES, P), DT_H)
    dp("w2t", (DEPTH, DTILES, P, 32, P), DT_H)
    dp("wproj", (P, KTILES, NCLS), DT_H)
    for nm, shp in [("bpe", (DIM,)),
                    ("lnp2g", (DIM,)), ("lnp2b", (DIM,)),
                    ("bqk", (DEPTH, 2 * DIM)), ("bv", (DEPTH, INNER)),
                    ("bov", (DEPTH, DIM)), ("b1v", (DEPTH, MLP)),
                    ("b2v", (DEPTH, DIM)), ("bprojv", (NCLS,))]:
        dp(nm, shp, DT_F)
    dp("out", (NCLS, TOK), DT_F, is_out=True)
    if dbg:
        dp("d_xemb", (P, DTILES, TOK), DT_R, is_out=True)
        dp("d_xa0", (P, DTILES, TOK), DT_H, is_out=True)
        dp("d_qf", (P, DTILES, 512), DT_H, is_out=True)
        dp("d_kf", (P, DTILES, 512), DT_H, is_out=True)
        dp("d_vt", (P, SEQT, HEADS, DHEAD), DT_H, is_out=True)
        dp("d_etr", (P, SEQT, 512), DT_H, is_out=True)
        dp("d_of", (P, DTILES, TOK), DT_H, is_out=True)
        for l in range(DEPTH):
            dp(f"d_xatt{l}", (P, DTILES, TOK), DT_R, is_out=True)
            dp(f"d_xffn{l}", (P, DTILES, TOK), DT_R, is_out=True)

    with tile.TileContext(nc) as tc:
        _emit(nc, tc, par, p2_trivial, v_bias_zero, qk_bias_zero, dbg)
    nc.compile()
    return nc


def _emit(nc, tc, par, p2_trivial, v_bias_zero, qk_bias_zero=True, dbg=False):
    import contextlib
    ctx = contextlib.ExitStack()
    with ctx:
        const = ctx.enter_context(tc.tile_pool(name="const", bufs=1))
        xpool = ctx.enter_context(tc.tile_pool(name="xpool", bufs=1))
        nrm = ctx.enter_context(tc.tile_pool(name="nrm", bufs=2))
        wsm = ctx.enter_context(tc.tile_pool(name="wsm", bufs=4))
        stats = ctx.enter_context(tc.tile_pool(name="stats", bufs=2))
        lnb = ctx.enter_context(tc.tile_pool(name="lnb", bufs=2))
        sm2 = ctx.enter_context(tc.tile_pool(name="sm2", bufs=4))
        atp = ctx.enter_context(tc.tile_pool(name="atp", bufs=3))
        etp = ctx.enter_context(tc.tile_pool(name="etp", bufs=1))
        psm = ctx.enter_context(tc.tile_pool(name="psm", bufs=4, space="PSUM"))
        pso = ctx.enter_context(tc.tile_pool(name="pso", bufs=2, space="PSUM"))
        pst = ctx.enter_context(tc.tile_pool(name="pst", bufs=1, space="PSUM"))

        ones_r = const.tile([P, 1], DT_R, name="ones_r")
        nc.vector.memset(ones_r.bitcast(mybir.dt.uint32), 0x3F800000)
        epst = const.tile([1, 1], DT_F, name="epst")
        nc.vector.memset(epst, EPS)
        ones_h = const.tile([P, 1], DT_H, name="ones_h")
        nc.vector.memset(ones_h.bitcast(mybir.dt.uint16), 0x3F80)

        def load_vec(nm, width):
            d = par[nm]
            if len(d.shape) == 1:
                tl = const.tile([P, width // P], DT_F, name=nm + "_t")
                nc.sync.dma_start(out=tl, in_=d.rearrange("(o p) -> p o", p=P))
            else:
                L = d.shape[0]
                tl = const.tile([P, L, width // P], DT_F, name=nm + "_t")
                nc.sync.dma_start(out=tl, in_=d.rearrange("l (o p) -> p l o", p=P))
            return tl

        bpe_t = load_vec("bpe", DIM)
        bqk_t = load_vec("bqk", 2 * DIM)
        bov_t = load_vec("bov", DIM)
        b1v_t = load_vec("b1v", MLP)
        b2v_t = load_vec("b2v", DIM)
        if not p2_trivial:
            lnp2g_t = load_vec("lnp2g", DIM)
            lnp2b_t = load_vec("lnp2b", DIM)
        bproj_t = const.tile([NCLS, 1], DT_F, name="bproj_t")
        nc.sync.dma_start(out=bproj_t,
                          in_=par["bprojv"].rearrange("(p o) -> p o", o=1))

        x = xpool.tile([P, DTILES, TOK], DT_R, name="x")

        # ---- one 512-wide LN half: stats -> mu/rstd -> bcast -> apply ----
        # views(d): [P,512] fp32(r) source; dst(d): [P,512] output slice.
        # gb: optional (g_fn, b_fn) per-partition affine (non-folded path).
        def ln_half(views, dst, ntiles, D, gb=None, hsrc=False):
            onev = ones_h if hsrc else ones_r
            sqdt = DT_H if hsrc else DT_R
            ps0 = pst.tile([1, 512], DT_F, name="ps0")
            ps1 = pst.tile([1, 512], DT_F, name="ps1")
            for d in range(ntiles):
                sq = sm2.tile([P, 512], sqdt, name="sq")
                nc.scalar.square(sq, views(d))
                nc.tensor.matmul(ps0, onev, views(d),
                                 start=(d == 0), stop=(d == ntiles - 1))
                nc.tensor.matmul(ps1, onev, sq,
                                 start=(d == 0), stop=(d == ntiles - 1))
            rows = stats.tile([1, 3, 512], DT_F, name="rows")
            mu, var, rstd = rows[:, 0, :], rows[:, 1, :], rows[:, 2, :]
            nc.vector.tensor_scalar(mu, ps0, 1.0 / D, None, OP.mult)
            nc.vector.tensor_scalar(var, ps1, 1.0 / D, None, OP.mult)
            nc.vector.tensor_mul(rstd, mu, mu)
            nc.vector.tensor_sub(var, var, rstd)
            nc.scalar.activation(var, var, FX.Sqrt, bias=epst, scale=1.0)
            with nc.allow_low_precision(reason="approx rstd for LN"):
                nc.vector.reciprocal_approx_fast(out=rstd, in_=var)
            mub = lnb.tile([P, 512], DT_F, name="mub")
            rsb = lnb.tile([P, 512], DT_F, name="rsb")
            nc.gpsimd.partition_broadcast(mub, mu)
            nc.gpsimd.partition_broadcast(rsb, rstd)
            for d in range(ntiles):
                t = sm2.tile([P, 512], DT_F, name="lnt")
                nc.vector.tensor_sub(t, views(d), mub)
                if gb is None:
                    nc.vector.tensor_mul(dst(d), t, rsb)
                else:
                    nc.vector.tensor_mul(t, t, rsb)
                    nc.vector.tensor_scalar(dst(d), t, gb[0](d), gb[1](d),
                                            OP.mult, OP.add)

        # =================== embedding ===================
        xin, band = par["xin"], par["band"]
        xa = nrm.tile([P, DTILES, TOK], DT_H, name="nrmbuf")   # lna(0) out
        with (
            tc.tile_pool(name="sfp", bufs=1) as sfp,
            tc.tile_pool(name="pnp", bufs=1) as pnp,
            tc.tile_pool(name="xtp", bufs=2) as xtp,
            tc.tile_pool(name="bnp", bufs=1) as bnp,
        ):
            bandt = bnp.tile([P, T // P, 3, P], DT_H, name="bandt")
            nc.sync.dma_start(out=bandt,
                              in_=band.rearrange("c s p q -> p c s q"))
            sfs, pns, xts = [], [], []

            def emit_xt(b):
                xt = xtp.tile([P, T // P, F], DT_H, name="xt")
                nc.sync.dma_start(
                    out=xt, in_=xin[b].rearrange("(kt p) f -> p kt f", p=P))
                xts.append(xt)

            def emit_band(b):
                xt = xts[b]
                sf = sfp.tile([P, 2, T], DT_H, name="sf")
                sfs.append(sf)
                for fh in range(2):
                    for g4 in range(T // 512):
                        pg = psm.tile([P, 512], DT_F, name="pmain")
                        for q in range(4):
                            ct = g4 * 4 + q
                            svals = [s for s in range(3)
                                     if 0 <= ct - 1 + s < T // P]
                            for si, s in enumerate(svals):
                                kt = ct - 1 + s
                                nc.tensor.matmul(
                                    pg[:, bass.ts(q, P)],
                                    xt[:, kt, bass.ts(fh, P)],
                                    bandt[:, ct, s, :],
                                    start=(si == 0),
                                    stop=(si == len(svals) - 1))
                        nc.vector.tensor_copy(sf[:, fh, bass.ts(g4, 512)], pg)

            def emit_lnp1(b):
                sf = sfs[b]

                def pview(pt):
                    i, fh = pt // 2, pt % 2
                    return sf[:, fh, :].rearrange(
                        "p (s four) -> p four s", four=PH)[:, i, :]

                pn = pnp.tile([P, 8, 512], DT_H, name="pn")
                pns.append(pn)
                ln_half(pview, lambda d: pn[:, d, :], 8, PATCH, hsrc=True)

            def emit_pe(b):
                pn = pns[b]
                for dt in range(DTILES):
                    wt = wsm.tile([P, KTILES, P], DT_H, name="wsm_t")
                    nc.sync.dma_start(out=wt, in_=par["wpe"][dt])
                    pq = psm.tile([P, 512], DT_F, name="pmain")
                    for kt in range(KTILES):
                        nc.tensor.matmul(pq, wt[:, kt, :], pn[:, kt, :],
                                         start=(kt == 0), stop=(kt == 7))
                    nc.scalar.activation(x[:, dt, bass.ts(b, 512)], pq,
                                         FX.Identity, bias=bpe_t[:, dt:dt + 1])

            def emit_lnp2(b):
                # in-place: x <- (x - mu) * rstd (stats read x pre-apply)
                sl = bass.ts(b, 512)
                gb = None
                if not p2_trivial:
                    gb = (lambda d: lnp2g_t[:, d:d + 1],
                          lambda d: lnp2b_t[:, d:d + 1])
                ln_half(lambda d: x[:, d, sl], lambda d: x[:, d, sl],
                        DTILES, DIM, gb=gb)

            def emit_lna0(b):
                sl = bass.ts(b, 512)
                ln_half(lambda d: x[:, d, sl], lambda d: xa[:, d, sl],
                        DTILES, DIM)

            emit_xt(0)
            emit_xt(1)
            emit_band(0)
            emit_lnp1(0)
            emit_pe(0)
            emit_band(1)
            emit_lnp2(0)
            emit_lnp1(1)
            emit_pe(1)
            emit_lna0(0)
            emit_lnp2(1)
            emit_lna0(1)
        if dbg:
            nc.sync.dma_start(out=par["d_xemb"][:, :, :], in_=x)
            nc.sync.dma_start(out=par["d_xa0"][:, :, :], in_=xa)

        # =================== transformer layers ===================
        for l in range(DEPTH):
            et_sb = etp.tile([P, SEQT, SEQ], DT_H, name="et_sb")
            nc.sync.dma_start(out=et_sb,
                              in_=par["etab"][l].rearrange("jt p i -> p jt i"))

            of = nrm.tile([P, DTILES, TOK], DT_H, name="nrmbuf")
            for b in range(BPC):
                tsl = bass.ts(b, 512)
                with tc.tile_pool(name=f"qkv{l}_{b}", bufs=1) as qkvp:
                    qf = qkvp.tile([P, DTILES, 512], DT_H, name="qf")
                    kf = qkvp.tile([P, DTILES, 512], DT_H, name="kf")
                    vt = qkvp.tile([P, SEQT, HEADS, DHEAD + 1], DT_H,
                                   name="vt")
                    nc.vector.memset(vt[:, :, :, DHEAD], 1.0)
                    # q, k feature-major (g_a and 1/sqrt(dhead) folded into W)
                    for c in range(2 * DTILES):
                        wt = wsm.tile([P, KTILES, P], DT_H, name="wsm_t")
                        nc.sync.dma_start(out=wt, in_=par["wqk_t"][l, c])
                        pq = psm.tile([P, 512], DT_F, name="pmain")
                        for kt in range(KTILES):
                            nc.tensor.matmul(pq, wt[:, kt, :], xa[:, kt, tsl],
                                             start=(kt == 0), stop=(kt == 7))
                        dst = qf if c < DTILES else kf
                        if qk_bias_zero:
                            nc.vector.tensor_copy(dst[:, c % DTILES, :], pq)
                        else:
                            nc.scalar.activation(dst[:, c % DTILES, :], pq,
                                                 FX.Identity,
                                                 bias=bqk_t[:, l, c:c + 1])
                    # v token-major with ones column per head
                    for nh in range(2):
                        for tth in range(2):
                            tts = (2 * tth, 2 * tth + 1)
                            pvs = [psm.tile([P, 512], DT_F, name="pmain")
                                   for _ in tts]
                            for kt in range(KTILES):
                                wv = wsm.tile([P, 512], DT_H, name="wsm_t")
                                nc.sync.dma_start(
                                    out=wv, in_=par["wv_t"][l, nh, kt])
                                for ti, tt in enumerate(tts):
                                    nc.tensor.matmul(
                                        pvs[ti],
                                        xa[:, kt, b * 512 + tt * P:
                                           b * 512 + (tt + 1) * P],
                                        wv,
                                        start=(kt == 0), stop=(kt == 7))
                            for ti, tt in enumerate(tts):
                                nc.vector.tensor_copy(
                                    vt[:, tt, nh * 8:(nh + 1) * 8, 0:DHEAD],
                                    pvs[ti].rearrange("p (h d) -> p h d",
                                                      h=8))

                    def emit_sc(dt):
                        etrA = atp.tile([P, SEQT, 512], DT_H, name="etr")
                        etrB = atp.tile([P, SEQT, 512], DT_H, name="etr")
                        if dbg and l == 0 and b == 0 and dt == 0:
                            nc.vector.memset(etrA, 0.0)
                        for jt in range(SEQT):
                            i0 = jt * P
                            jts = bass.ts(jt, P)
                            scA = psm.tile([P, 512], DT_F, name="pmain")
                            scB = psm.tile([P, 512], DT_F, name="pmain")
                            nc.tensor.matmul(
                                scA[:, i0:], kf[0:64, dt, jts],
                                qf[0:64, dt, i0:],
                                start=True, stop=True, tile_position=(0, 0))
                            nc.tensor.matmul(
                                scB[:, i0:], kf[64:128, dt, jts],
                                qf[64:128, dt, i0:],
                                start=True, stop=True, tile_position=(64, 0))
                            for sc, etr in ((scA, etrA), (scB, etrB)):
                                ex = sm2.tile([P, 512], DT_F, name="ex")
                                nc.scalar.activation(ex[:, i0:], sc[:, i0:],
                                                     FX.Exp)
                                nc.vector.tensor_mul(
                                    etr[:, jt, i0:], ex[:, i0:],
                                    et_sb[:, jt, i0:])
                        if dbg and l == 0 and b == 0 and dt == 0:
                            nc.sync.dma_start(out=par["d_etr"][:, :, :],
                                              in_=etrA)
                        return etrA, etrB

                    def emit_probs(dt, etrA, etrB):
                        hA, hB = 2 * dt, 2 * dt + 1
                        for hd, etr in ((hA, etrA), (hB, etrB)):
                            ot = pso.tile([P, 512], DT_F, name="ot")
                            for jt in range(SEQT):
                                i0 = jt * P
                                nc.tensor.matmul(
                                    ot[0:DHEAD + 1, i0:],
                                    vt[:, jt, hd, :],
                                    etr[:, jt, i0:],
                                    start=(jt == 0), stop=(jt == SEQT - 1))
                            # den sits at PSUM row 64; DVE can read it in
                            # place (no partition shift), the SBUF->SBUF DMA
                            # moves it to partition 0 for recip+broadcast.
                            dsb = stats.tile([65, 512], DT_F, name="dsb")
                            nc.vector.tensor_copy(dsb[64:65, :],
                                                  ot[64:65, :])
                            den0 = stats.tile([1, 512], DT_F, name="den0")
                            nc.sync.dma_start(out=den0, in_=dsb[64:65, :])
                            adr1 = stats.tile([1, 512], DT_F, name="adr1")
                            nc.vector.reciprocal_approx_fast(
                                out=adr1, in_=den0)
                            adrb = stats.tile([64, 512], DT_F, name="adrb")
                            nc.gpsimd.partition_broadcast(adrb, adr1)
                            if hd % 2 == 0:
                                nc.vector.tensor_mul(
                                    of[0:64, dt, tsl], ot[0:64, :], adrb)
                            else:
                                # DVE can't shift partitions: normalize at
                                # base 0, DMA-copy up to partitions 64:128.
                                otmp = sm2.tile([64, 512], DT_H, name="otmp")
                                nc.vector.tensor_mul(otmp, ot[0:64, :], adrb)
                                nc.sync.dma_start(
                                    out=of[64:128, dt, tsl], in_=otmp)

                    prev = None
                    for dt in range(DTILES):
                        cur = emit_sc(dt)
                        if prev is not None:
                            emit_probs(dt - 1, *prev)
                        prev = cur
                    emit_probs(DTILES - 1, *prev)
                    if dbg and l == 0 and b == 0:
                        nc.sync.dma_start(out=par["d_qf"][:, :, :], in_=qf)
                        nc.sync.dma_start(out=par["d_kf"][:, :, :], in_=kf)
                        nc.sync.dma_start(out=par["d_vt"][:, :, :, :],
                                          in_=vt)

            xf = nrm.tile([P, DTILES, TOK], DT_H, name="nrmbuf")
            for b in range(BPC):
                tsl = bass.ts(b, 512)
                for dt in range(DTILES):
                    wt = wsm.tile([P, KTILES, P], DT_H, name="wsm_t")
                    nc.sync.dma_start(out=wt, in_=par["wot"][l, dt])
                    pq = psm.tile([P, 512], DT_F, name="pmain")
                    for kt in range(KTILES):
                        nc.tensor.matmul(
                            pq, wt[:, kt, :], of[:, kt, tsl],
                            start=(kt == 0), stop=(kt == 7))
                    nc.scalar.activation(pq, pq, FX.Identity,
                                         bias=bov_t[:, l, dt:dt + 1])
                    nc.vector.tensor_add(
                        x[:, dt, tsl], pq, x[:, dt, tsl])
                # lnf for this half while the other half's wo / ffn runs
                ln_half(lambda d: x[:, d, tsl], lambda d: xf[:, d, tsl],
                        DTILES, DIM)
            if dbg:
                nc.sync.dma_start(out=par[f"d_xatt{l}"][:, :, :], in_=x)
                if l == 0:
                    nc.sync.dma_start(out=par["d_of"][:, :, :], in_=of)

            if l < DEPTH - 1:
                xa = nrm.tile([P, DTILES, TOK], DT_H, name="nrmbuf")
            else:
                xa = nrm.tile([P, DTILES, TOK], DT_H, name="nrmbuf")  # ho
            for th in range(2):
                tsl = bass.ts(th, 512)
                with tc.tile_pool(name=f"h1p{l}_{th}", bufs=1) as h1p:
                    h1r = h1p.tile([P, MTILES, 512], DT_H, name="h1r")
                    for mt in range(MTILES):
                        wt = wsm.tile([P, KTILES, P], DT_H, name="wsm_t")
                        nc.sync.dma_start(out=wt, in_=par["w1t"][l, mt])
                        pq = psm.tile([P, 512], DT_F, name="pmain")
                        for kt in range(KTILES):
                            nc.tensor.matmul(pq, wt[:, kt, :], xf[:, kt, tsl],
                                             start=(kt == 0), stop=(kt == 7))
                        nc.scalar.activation(
                            h1r[:, mt, :], pq, FX.Gelu,
                            bias=b1v_t[:, l, mt:mt + 1], scale=1.0)
                    for dt in range(DTILES):
                        pq = psm.tile([P, 512], DT_F, name="pmain")
                        for kq in range(4):
                            wt = wsm.tile([P, 8, P], DT_H, name="wsm_t")
                            nc.sync.dma_start(
                                out=wt,
                                in_=par["w2t"][l, dt, :,
                                               kq * 8:(kq + 1) * 8, :])
                            for k2 in range(8):
                                kt = kq * 8 + k2
                                nc.tensor.matmul(
                                    pq, wt[:, k2, :], h1r[:, kt, :],
                                    start=(kt == 0), stop=(kt == 31))
                        nc.scalar.activation(pq, pq, FX.Identity,
                                             bias=b2v_t[:, l, dt:dt + 1])
                        nc.vector.tensor_add(
                            x[:, dt, tsl], pq, x[:, dt, tsl])
                # lna of next layer (or final LN) for this half
                ln_half(lambda d: x[:, d, tsl], lambda d: xa[:, d, tsl],
                        DTILES, DIM)
            if dbg:
                nc.sync.dma_start(out=par[f"d_xffn{l}"][:, :, :], in_=x)

        # =================== head (xa == ho, lno folded into wproj) =======
        wt = wsm.tile([P, KTILES, NCLS], DT_H, name="wproj_t")
        nc.sync.dma_start(out=wt, in_=par["wproj"][:, :, :])
        out_sb = const.tile([NCLS, TOK], DT_F, name="out_sb")
        for th in range(2):
            pq = psm.tile([P, 512], DT_F, name="pmain")
            for kt in range(KTILES):
                nc.tensor.matmul(pq[0:NCLS, :], wt[:, kt, :],
                                 xa[:, kt, bass.ts(th, 512)],
                                 start=(kt == 0), stop=(kt == 7))
            nc.scalar.activation(out_sb[:, bass.ts(th, 512)], pq[0:NCLS, :],
                                 FX.Identity, bias=bproj_t)
        nc.sync.dma_start(out=par["out"][:, :], in_=out_sb)


# ============================================================
# host side
# ============================================================

_NC_CACHE = {}


def _bf16(a):
    import ml_dtypes
    return np.ascontiguousarray(a.astype(ml_dtypes.bfloat16))


def _pack_qk(w):      # [D, DIM, 2048] -> [D, 16, P, 8, P]
    v = w.reshape(DEPTH, 8, P, 16, P).transpose(0, 3, 2, 1, 4)
    return _bf16(v)


def _pack_v(w):       # [D, DIM, 1024] -> [D, 2, 8, P, 512]
    v = w.reshape(DEPTH, 8, P, 2, 512).transpose(0, 3, 1, 2, 4)
    return _bf16(v)


def _pack_kxm(w):     # [D, K, M] -> [D, M//P, P, K//P, P]
    D, K, M = w.shape
    v = w.reshape(D, K // P, P, M // P, P).transpose(0, 3, 2, 1, 4)
    return _bf16(v)


def _pack_w2(w):      # [D, 4096, 1024] -> [D, 8, P, 32, P]
    v = w.reshape(DEPTH, 32, P, 8, P).transpose(0, 3, 2, 1, 4)
    return _bf16(v)


def _host_band():
    tt = np.arange(KSIZE, dtype=np.float64)
    kern = np.exp(-0.5 * ((tt - (KSIZE - 1) / 2.0) / SIGMA) ** 2)
    kern = (kern / kern.sum()).astype(np.float32)
    pad_l = (KSIZE - 1) // 2  # 9
    nt = T // P
    bandc = np.zeros((nt, 3, P, P), dtype=np.float32)
    for ct in range(nt):
        for s in range(3):
            kt = ct - 1 + s
            if not (0 <= kt < nt):
                continue
            rows = np.arange(kt * P, (kt + 1) * P)
            cols = np.arange(ct * P, (ct + 1) * P)
            d = rows[:, None] - cols[None, :] + pad_l
            m = (d >= 0) & (d < KSIZE)
            blk = np.zeros((P, P), np.float32)
            blk[m] = kern[d[m]]
            bandc[ct, s] = blk
    return bandc


def _host_etab(rel_tab):
    i = np.arange(SEQ)
    j = i[:, None]
    rel = np.clip(i[None, :] - j, -(MAXREL - 1), MAXREL - 1) + MAXREL - 1
    et = np.zeros((DEPTH, SEQ, SEQ), dtype=np.float32)
    for l in range(DEPTH):
        e = np.exp(rel_tab[l][rel])
        e[j > i[None, :]] = 0.0
        et[l] = e
    return et.reshape(DEPTH, SEQT, P, SEQ)


def kernel(**inputs):
    global _NC_CACHE

    f32 = lambda a: np.ascontiguousarray(np.asarray(a, dtype=np.float32))
    dscale = float(DHEAD) ** -0.5

    wqkv = f32(inputs["Wqkv"])                      # [D, DIM, 3072]
    g_a, b_a = f32(inputs["ln_a_g"]), f32(inputs["ln_a_b"])   # [D, DIM]
    wqk = wqkv[:, :, :2 * DIM] * g_a[:, :, None]
    wqk[:, :, DIM:] *= dscale
    bqk = np.einsum("lk,lkm->lm", b_a, wqkv[:, :, :2 * DIM])
    bqk[:, DIM:] *= dscale
    wv = wqkv[:, :, 2 * DIM:] * g_a[:, :, None]
    bv = np.einsum("lk,lkm->lm", b_a, wqkv[:, :, 2 * DIM:])

    w1 = f32(inputs["W1"])
    g_f, b_f = f32(inputs["ln_f_g"]), f32(inputs["ln_f_b"])
    w1g = w1 * g_f[:, :, None]
    b1f = f32(inputs["b1"]) + np.einsum("lk,lkm->lm", b_f, w1)

    wpe = f32(inputs["W_pe"])
    g_p1, b_p1 = f32(inputs["ln_p1_g"]), f32(inputs["ln_p1_b"])
    wpeg = wpe * g_p1[:, None]
    bpef = f32(inputs["b_pe"]) + b_p1 @ wpe
    # [K,M] -> [M//P, P(k), K//P, P(m)]
    wpe_p = _bf16(wpeg.reshape(KTILES, P, DTILES, P).transpose(2, 1, 0, 3))

    wproj = f32(inputs["Wproj"])
    g_o, b_o = f32(inputs["ln_o_g"]), f32(inputs["ln_o_b"])
    wprojg = wproj * g_o[:, None]
    bprojf = f32(inputs["bproj"]) + b_o @ wproj
    wproj_p = _bf16(wprojg.reshape(KTILES, P, NCLS).transpose(1, 0, 2))

    p2g, p2b = f32(inputs["ln_p2_g"]), f32(inputs["ln_p2_b"])
    p2_trivial = bool(np.all(p2g == 1.0) and np.all(p2b == 0.0))
    v_bias_zero = bool(np.all(bv == 0.0))
    assert v_bias_zero, "nonzero folded v bias not supported in this build"
    qk_bias_zero = bool(np.all(bqk == 0.0))

    key = (p2_trivial, v_bias_zero, qk_bias_zero)
    if key not in _NC_CACHE:
        _NC_CACHE[key] = build_nc(*key)
    nc = _NC_CACHE[key]

    shared = {
        "band": _bf16(_host_band()),
        "etab": _bf16(_host_etab(f32(inputs["rel_tab"]))),
        "wpe": wpe_p,
        "wqk_t": _pack_qk(wqk),
        "wv_t": _pack_v(wv),
        "wot": _pack_kxm(f32(inputs["Wo"])),
        "w1t": _pack_kxm(w1g),
        "w2t": _pack_w2(f32(inputs["W2"])),
        "wproj": wproj_p,
        "bpe": bpef,
        "lnp2g": p2g, "lnp2b": p2b,
        "bqk": np.ascontiguousarray(bqk, dtype=np.float32),
        "bv": np.ascontiguousarray(bv, dtype=np.float32),
        "bov": f32(inputs["bo"]), "b1v": np.ascontiguousarray(b1f, np.float32),
        "b2v": f32(inputs["b2"]),
        "bprojv": np.ascontiguousarray(bprojf, np.float32),
    }
    xfull = f32(inputs["neuralInput"])
    in_maps = []
    for c in range(NCORES):
        m = dict(shared)
        m["xin"] = _bf16(xfull[c * BPC:(c + 1) * BPC])
        in_maps.append(m)

    import os
    trace = bool(os.environ.get("BIT_TRACE"))
    res = run_bass_kernel_spmd(nc, in_maps, list(range(NCORES)), trace=trace)
    if trace:
        globals()["LAST_RESULT"] = res
    outs = []
    for c in range(NCORES):
        o = res.results[c]["out"]              # [NCLS, TOK]
        o = o.reshape(NCLS, BPC, SEQ).transpose(1, 2, 0)
        outs.append(o)
    return np.concatenate(outs, axis=0).astype(np.float32)


# revision 22
# speedup vs baseline: 1.0206x; 1.0206x over previous
"""Trainium2 Bass kernel for nn_BiT_Phoneme (dense transformer).

Data-parallel: 16 batch elems / 8 cores = 2 per core; each core runs the
full network on its 2 sequences (1024 "tokens"). Activations are kept
feature-major ([dim on partitions, tokens on free]) so matmuls chain
without transposes. The gaussian time-smoothing + patchify are a banded
matmul with a host-precomputed band matrix.

v2 redesign (vs baseline at 4.30 ms):
- LayerNorm gains/biases are folded into the *following* weight matrix on
  the host (W' = g*W, bias' += b@W), so the on-device LN apply is just
  (x - mu)*rstd -> bf16: 2 DVE ops/tile instead of 3, and the affine
  params never touch the device.
- rstd = reciprocal_approx_fast(sqrt(var+eps)) instead of the exact
  6 us/call nc.vector.reciprocal.
- mu/rstd partition-broadcasts run on GpSimd (partition_broadcast), not
  the PE, so LN never blocks the in-order PE queue.
- LNs are computed per 512-token half, and emission is dataflow-ordered:
  each half's stats+apply are emitted immediately after the phase that
  produces that half (wo b0 -> lnf h0 -> wo b1 -> lnf h1 -> ffn th0 ->
  lna' h0 -> ffn th1 -> lna' h1 -> next layer). The PE stays busy with
  the other half's matmuls while DVE/Scalar/GpSimd do the LN.
- Softmax denominators come free from a 65th all-ones column appended to
  each head's V block (probs@V matmul row 64 = sum of probs), removing
  768 M=1 PE matmuls.
- Score matmuls (K=64) for head pairs are emitted adjacently with
  explicit tile_position (0,0)/(64,0) so the two 64-row matmuls run
  concurrently in the PE array.
- q/k/v/wo/ffn drains moved to the Scalar engine (activation with
  per-partition bias) to balance DVE load.
Matmul dtypes: weights/activations bf16 (x residual stream fp32r).
"""

import numpy as np

import concourse.bass as bass
import concourse.mybir as mybir
import concourse.tile as tile
from concourse import bacc
from concourse.bass_utils import run_bass_kernel_spmd

B, T, F = 16, 2048, 256
PH = 4
PATCH = 1024
DIM = 1024
DEPTH = 6
HEADS, DHEAD = 16, 64
INNER = 1024
MLP = 4096
NCLS = 41
MAXREL = 200
KSIZE, SIGMA = 20, 2.0
EPS = 1e-5
SEQ = T // PH              # 512
NCORES = 8
BPC = B // NCORES          # 2
TOK = BPC * SEQ            # 1024
P = 128

DT_R = mybir.dt.float32r
DT_F = mybir.dt.float32
DT_H = mybir.dt.bfloat16
FX = mybir.ActivationFunctionType
OP = mybir.AluOpType

DTILES = DIM // P          # 8
KTILES = DIM // P          # 8
MTILES = MLP // P          # 32
SEQT = SEQ // P            # 4


def build_nc(p2_trivial, v_bias_zero, qk_bias_zero=True, dbg=False):
    nc = bacc.Bacc(None, target_bir_lowering=False)

    par = {}
    def dp(name, shape, dtype, is_out=False):
        par[name] = nc.declare_dram_parameter(name, list(shape), dtype, isOutput=is_out)
        return par[name]

    dp("xin", (BPC, T, F), DT_H)
    dp("band", (T // P, 3, P, P), DT_H)
    dp("etab", (DEPTH, SEQT, P, SEQ), DT_H)
    dp("wpe", (DTILES, P, KTILES, P), DT_H)
    dp("wqk_t", (DEPTH, 16, P, KTILES, P), DT_H)
    dp("wv_t", (DEPTH, 2, KTILES, P, 512), DT_H)
    dp("wot", (DEPTH, DTILES, P, KTILES, P), DT_H)
    dp("w1t", (DEPTH, MTILES, P, KTILES, P), DT_H)
    dp("w2t", (DEPTH, DTILES, P, 32, P), DT_H)
    dp("wproj", (P, KTILES, NCLS), DT_H)
    for nm, shp in [("bpe", (DIM,)),
                    ("lnp2g", (DIM,)), ("lnp2b", (DIM,)),
                    ("bqk", (DEPTH, 2 * DIM)), ("bv", (DEPTH, INNER)),
                    ("bov", (DEPTH, DIM)), ("b1v", (DEPTH, MLP)),
                    ("b2v", (DEPTH, DIM)), ("bprojv", (NCLS,))]:
        dp(nm, shp, DT_F)
    dp("out", (NCLS, TOK), DT_F, is_out=True)
    if dbg:
        dp("d_xemb", (P, DTILES, TOK), DT_R, is_out=True)
        dp("d_xa0", (P, DTILES, TOK), DT_H, is_out=True)
        dp("d_qf", (P, DTILES, 512), DT_H, is_out=True)
        dp("d_kf", (P, DTILES, 512), DT_H, is_out=True)
        dp("d_vt", (P, SEQT, HEADS, DHEAD), DT_H, is_out=True)
        dp("d_etr", (P, SEQT, 512), DT_H, is_out=True)
        dp("d_of", (P, DTILES, TOK), DT_H, is_out=True)
        for l in range(DEPTH):
            dp(f"d_xatt{l}", (P, DTILES, TOK), DT_R, is_out=True)
            dp(f"d_xffn{l}", (P, DTILES, TOK), DT_R, is_out=True)

    with tile.TileContext(nc) as tc:
        _emit(nc, tc, par, p2_trivial, v_bias_zero, qk_bias_zero, dbg)
    nc.compile()
    return nc


def _emit(nc, tc, par, p2_trivial, v_bias_zero, qk_bias_zero=True, dbg=False):
    import contextlib
    ctx = contextlib.ExitStack()
    with ctx:
        const = ctx.enter_context(tc.tile_pool(name="const", bufs=1))
        xpool = ctx.enter_context(tc.tile_pool(name="xpool", bufs=1))
        nrm = ctx.enter_context(tc.tile_pool(name="nrm", bufs=2))
        wsm = ctx.enter_context(tc.tile_pool(name="wsm", bufs=4))
        stats = ctx.enter_context(tc.tile_pool(name="stats", bufs=2))
        lnb = ctx.enter_context(tc.tile_pool(name="lnb", bufs=2))
        sm2 = ctx.enter_context(tc.tile_pool(name="sm2", bufs=4))
        atp = ctx.enter_context(tc.tile_pool(name="atp", bufs=2))
        etp = ctx.enter_context(tc.tile_pool(name="etp", bufs=1))
        psm = ctx.enter_context(tc.tile_pool(name="psm", bufs=4, space="PSUM"))
        pso = ctx.enter_context(tc.tile_pool(name="pso", bufs=2, space="PSUM"))
        pst = ctx.enter_context(tc.tile_pool(name="pst", bufs=1, space="PSUM"))

        ones_r = const.tile([P, 1], DT_R, name="ones_r")
        nc.vector.memset(ones_r.bitcast(mybir.dt.uint32), 0x3F800000)
        epst = const.tile([1, 1], DT_F, name="epst")
        nc.vector.memset(epst, EPS)
        ones_h = const.tile([P, 1], DT_H, name="ones_h")
        nc.vector.memset(ones_h.bitcast(mybir.dt.uint16), 0x3F80)

        def load_vec(nm, width):
            d = par[nm]
            if len(d.shape) == 1:
                tl = const.tile([P, width // P], DT_F, name=nm + "_t")
                nc.sync.dma_start(out=tl, in_=d.rearrange("(o p) -> p o", p=P))
            else:
                L = d.shape[0]
                tl = const.tile([P, L, width // P], DT_F, name=nm + "_t")
                nc.sync.dma_start(out=tl, in_=d.rearrange("l (o p) -> p l o", p=P))
            return tl

        bpe_t = load_vec("bpe", DIM)
        bqk_t = load_vec("bqk", 2 * DIM)
        bov_t = load_vec("bov", DIM)
        b1v_t = load_vec("b1v", MLP)
        b2v_t = load_vec("b2v", DIM)
        if not p2_trivial:
            lnp2g_t = load_vec("lnp2g", DIM)
            lnp2b_t = load_vec("lnp2b", DIM)
        bproj_t = const.tile([NCLS, 1], DT_F, name="bproj_t")
        nc.sync.dma_start(out=bproj_t,
                          in_=par["bprojv"].rearrange("(p o) -> p o", o=1))

        x = xpool.tile([P, DTILES, TOK], DT_R, name="x")

        # ---- one 512-wide LN half: stats -> mu/rstd -> bcast -> apply ----
        # views(d): [P,512] fp32(r) source; dst(d): [P,512] output slice.
        # gb: optional (g_fn, b_fn) per-partition affine (non-folded path).
        def ln_half(views, dst, ntiles, D, gb=None, hsrc=False):
            onev = ones_h if hsrc else ones_r
            sqdt = DT_H if hsrc else DT_R
            ps0 = pst.tile([1, 512], DT_F, name="ps0")
            ps1 = pst.tile([1, 512], DT_F, name="ps1")
            for d in range(ntiles):
                sq = sm2.tile([P, 512], sqdt, name="sq")
                nc.scalar.square(sq, views(d))
                nc.tensor.matmul(ps0, onev, views(d),
                                 start=(d == 0), stop=(d == ntiles - 1))
                nc.tensor.matmul(ps1, onev, sq,
                                 start=(d == 0), stop=(d == ntiles - 1))
            rows = stats.tile([1, 3, 512], DT_F, name="rows")
            mu, var, rstd = rows[:, 0, :], rows[:, 1, :], rows[:, 2, :]
            nc.vector.tensor_scalar(mu, ps0, 1.0 / D, None, OP.mult)
            nc.vector.tensor_scalar(var, ps1, 1.0 / D, None, OP.mult)
            nc.vector.tensor_mul(rstd, mu, mu)
            nc.vector.tensor_sub(var, var, rstd)
            nc.scalar.activation(var, var, FX.Sqrt, bias=epst, scale=1.0)
            with nc.allow_low_precision(reason="approx rstd for LN"):
                nc.vector.reciprocal_approx_fast(out=rstd, in_=var)
            mub = lnb.tile([P, 512], DT_F, name="mub")
            rsb = lnb.tile([P, 512], DT_F, name="rsb")
            nc.gpsimd.partition_broadcast(mub, mu)
            nc.gpsimd.partition_broadcast(rsb, rstd)
            for d in range(ntiles):
                t = sm2.tile([P, 512], DT_F, name="lnt")
                nc.vector.tensor_sub(t, views(d), mub)
                if gb is None:
                    nc.vector.tensor_mul(dst(d), t, rsb)
                else:
                    nc.vector.tensor_mul(t, t, rsb)
                    nc.vector.tensor_scalar(dst(d), t, gb[0](d), gb[1](d),
                                            OP.mult, OP.add)

        # =================== embedding ===================
        xin, band = par["xin"], par["band"]
        xa = nrm.tile([P, DTILES, TOK], DT_H, name="nrmbuf")   # lna(0) out
        with (
            tc.tile_pool(name="sfp", bufs=1) as sfp,
            tc.tile_pool(name="pnp", bufs=1) as pnp,
            tc.tile_pool(name="xtp", bufs=2) as xtp,
            tc.tile_pool(name="bnp", bufs=1) as bnp,
        ):
            bandt = bnp.tile([P, T // P, 3, P], DT_H, name="bandt")
            nc.sync.dma_start(out=bandt,
                              in_=band.rearrange("c s p q -> p c s q"))
            sfs, pns, xts = [], [], []

            def emit_xt(b):
                xt = xtp.tile([P, T // P, F], DT_H, name="xt")
                nc.sync.dma_start(
                    out=xt, in_=xin[b].rearrange("(kt p) f -> p kt f", p=P))
                xts.append(xt)

            def emit_band(b):
                xt = xts[b]
                sf = sfp.tile([P, 2, T], DT_H, name="sf")
                sfs.append(sf)
                for fh in range(2):
                    for g4 in range(T // 512):
                        pg = psm.tile([P, 512], DT_F, name="pmain")
                        for q in range(4):
                            ct = g4 * 4 + q
                            svals = [s for s in range(3)
                                     if 0 <= ct - 1 + s < T // P]
                            for si, s in enumerate(svals):
                                kt = ct - 1 + s
                                nc.tensor.matmul(
                                    pg[:, bass.ts(q, P)],
                                    xt[:, kt, bass.ts(fh, P)],
                                    bandt[:, ct, s, :],
                                    start=(si == 0),
                                    stop=(si == len(svals) - 1))
                        nc.vector.tensor_copy(sf[:, fh, bass.ts(g4, 512)], pg)

            def emit_lnp1(b):
                sf = sfs[b]

                def pview(pt):
                    i, fh = pt // 2, pt % 2
                    return sf[:, fh, :].rearrange(
                        "p (s four) -> p four s", four=PH)[:, i, :]

                pn = pnp.tile([P, 8, 512], DT_H, name="pn")
                pns.append(pn)
                ln_half(pview, lambda d: pn[:, d, :], 8, PATCH, hsrc=True)

            def emit_pe(b):
                pn = pns[b]
                for dt in range(DTILES):
                    wt = wsm.tile([P, KTILES, P], DT_H, name="wsm_t")
                    nc.sync.dma_start(out=wt, in_=par["wpe"][dt])
                    pq = psm.tile([P, 512], DT_F, name="pmain")
                    for kt in range(KTILES):
                        nc.tensor.matmul(pq, wt[:, kt, :], pn[:, kt, :],
                                         start=(kt == 0), stop=(kt == 7))
                    nc.scalar.activation(x[:, dt, bass.ts(b, 512)], pq,
                                         FX.Identity, bias=bpe_t[:, dt:dt + 1])

            def emit_lnp2(b):
                # in-place: x <- (x - mu) * rstd (stats read x pre-apply)
                sl = bass.ts(b, 512)
                gb = None
                if not p2_trivial:
                    gb = (lambda d: lnp2g_t[:, d:d + 1],
                          lambda d: lnp2b_t[:, d:d + 1])
                ln_half(lambda d: x[:, d, sl], lambda d: x[:, d, sl],
                        DTILES, DIM, gb=gb)

            def emit_lna0(b):
                sl = bass.ts(b, 512)
                ln_half(lambda d: x[:, d, sl], lambda d: xa[:, d, sl],
                        DTILES, DIM)

            emit_xt(0)
            emit_xt(1)
            emit_band(0)
            emit_lnp1(0)
            emit_pe(0)
            emit_band(1)
            emit_lnp2(0)
            emit_lnp1(1)
            emit_pe(1)
            emit_lna0(0)
            emit_lnp2(1)
            emit_lna0(1)
        if dbg:
            nc.sync.dma_start(out=par["d_xemb"][:, :, :], in_=x)
            nc.sync.dma_start(out=par["d_xa0"][:, :, :], in_=xa)

        # =================== transformer layers ===================
        for l in range(DEPTH):
            et_sb = etp.tile([P, SEQT, SEQ], DT_H, name="et_sb")
            nc.sync.dma_start(out=et_sb,
                              in_=par["etab"][l].rearrange("jt p i -> p jt i"))

            of = nrm.tile([P, DTILES, TOK], DT_H, name="nrmbuf")
            for b in range(BPC):
                tsl = bass.ts(b, 512)
                with tc.tile_pool(name=f"qkv{l}_{b}", bufs=1) as qkvp:
                    qf = qkvp.tile([P, DTILES, 512], DT_H, name="qf")
                    kf = qkvp.tile([P, DTILES, 512], DT_H, name="kf")
                    vt = qkvp.tile([P, SEQT, HEADS, DHEAD + 1], DT_H,
                                   name="vt")
                    nc.vector.memset(vt[:, :, :, DHEAD], 1.0)
                    # q, k feature-major (g_a and 1/sqrt(dhead) folded into W)
                    for c in range(2 * DTILES):
                        wt = wsm.tile([P, KTILES, P], DT_H, name="wsm_t")
                        nc.sync.dma_start(out=wt, in_=par["wqk_t"][l, c])
                        pq = psm.tile([P, 512], DT_F, name="pmain")
                        for kt in range(KTILES):
                            nc.tensor.matmul(pq, wt[:, kt, :], xa[:, kt, tsl],
                                             start=(kt == 0), stop=(kt == 7))
                        dst = qf if c < DTILES else kf
                        nc.scalar.activation(dst[:, c % DTILES, :], pq,
                                             FX.Identity,
                                             bias=bqk_t[:, l, c:c + 1])
                    # v token-major with ones column per head
                    for nh in range(2):
                        for tth in range(2):
                            tts = (2 * tth, 2 * tth + 1)
                            pvs = [psm.tile([P, 512], DT_F, name="pmain")
                                   for _ in tts]
                            for kt in range(KTILES):
                                wv = wsm.tile([P, 512], DT_H, name="wsm_t")
                                nc.sync.dma_start(
                                    out=wv, in_=par["wv_t"][l, nh, kt])
                                for ti, tt in enumerate(tts):
                                    nc.tensor.matmul(
                                        pvs[ti],
                                        xa[:, kt, b * 512 + tt * P:
                                           b * 512 + (tt + 1) * P],
                                        wv,
                                        start=(kt == 0), stop=(kt == 7))
                            for ti, tt in enumerate(tts):
                                nc.scalar.activation(
                                    vt[:, tt, nh * 8:(nh + 1) * 8, 0:DHEAD],
                                    pvs[ti].rearrange("p (h d) -> p h d",
                                                      h=8),
                                    FX.Identity)

                    for dt in range(DTILES):
                        hA, hB = 2 * dt, 2 * dt + 1
                        etrA = atp.tile([P, SEQT, 512], DT_H, name="etr")
                        etrB = atp.tile([P, SEQT, 512], DT_H, name="etr")
                        if dbg and l == 0 and b == 0 and dt == 0:
                            nc.vector.memset(etrA, 0.0)
                        for jt in range(SEQT):
                            i0 = jt * P
                            jts = bass.ts(jt, P)
                            scA = psm.tile([P, 512], DT_F, name="pmain")
                            scB = psm.tile([P, 512], DT_F, name="pmain")
                            nc.tensor.matmul(
                                scA[:, i0:], kf[0:64, dt, jts],
                                qf[0:64, dt, i0:],
                                start=True, stop=True, tile_position=(0, 0))
                            nc.tensor.matmul(
                                scB[:, i0:], kf[64:128, dt, jts],
                                qf[64:128, dt, i0:],
                                start=True, stop=True, tile_position=(64, 0))
                            for sc, etr in ((scA, etrA), (scB, etrB)):
                                ex = sm2.tile([P, 512], DT_F, name="ex")
                                nc.scalar.activation(ex[:, i0:], sc[:, i0:],
                                                     FX.Exp)
                                nc.vector.tensor_mul(
                                    etr[:, jt, i0:], ex[:, i0:],
                                    et_sb[:, jt, i0:])
                        if dbg and l == 0 and b == 0 and dt == 0:
                            nc.sync.dma_start(out=par["d_etr"][:, :, :],
                                              in_=etrA)
                        for hd, etr in ((hA, etrA), (hB, etrB)):
                            ot = pso.tile([P, 512], DT_F, name="ot")
                            for jt in range(SEQT):
                                i0 = jt * P
                                nc.tensor.matmul(
                                    ot[0:DHEAD + 1, i0:],
                                    vt[:, jt, hd, :],
                                    etr[:, jt, i0:],
                                    start=(jt == 0), stop=(jt == SEQT - 1))
                            # den sits at PSUM row 64; DVE can read it in
                            # place (no partition shift), the SBUF->SBUF DMA
                            # moves it to partition 0 for recip+broadcast.
                            dsb = stats.tile([65, 512], DT_F, name="dsb")
                            nc.vector.tensor_copy(dsb[64:65, :],
                                                  ot[64:65, :])
                            den0 = stats.tile([1, 512], DT_F, name="den0")
                            nc.sync.dma_start(out=den0, in_=dsb[64:65, :])
                            adr1 = stats.tile([1, 512], DT_F, name="adr1")
                            nc.vector.reciprocal_approx_fast(
                                out=adr1, in_=den0)
                            adrb = stats.tile([64, 512], DT_F, name="adrb")
                            nc.gpsimd.partition_broadcast(adrb, adr1)
                            if hd % 2 == 0:
                                nc.vector.tensor_mul(
                                    of[0:64, dt, tsl], ot[0:64, :], adrb)
                            else:
                                # DVE can't shift partitions: normalize at
                                # base 0, DMA-copy up to partitions 64:128.
                                otmp = sm2.tile([64, 512], DT_H, name="otmp")
                                nc.vector.tensor_mul(otmp, ot[0:64, :], adrb)
                                nc.sync.dma_start(
                                    out=of[64:128, dt, tsl], in_=otmp)
                    if dbg and l == 0 and b == 0:
                        nc.sync.dma_start(out=par["d_qf"][:, :, :], in_=qf)
                        nc.sync.dma_start(out=par["d_kf"][:, :, :], in_=kf)
                        nc.sync.dma_start(out=par["d_vt"][:, :, :, :],
                                          in_=vt)

            xf = nrm.tile([P, DTILES, TOK], DT_H, name="nrmbuf")
            for b in range(BPC):
                tsl = bass.ts(b, 512)
                for dt in range(DTILES):
                    wt = wsm.tile([P, KTILES, P], DT_H, name="wsm_t")
                    nc.sync.dma_start(out=wt, in_=par["wot"][l, dt])
                    pq = psm.tile([P, 512], DT_F, name="pmain")
                    for kt in range(KTILES):
                        nc.tensor.matmul(
                            pq, wt[:, kt, :], of[:, kt, tsl],
                            start=(kt == 0), stop=(kt == 7))
                    nc.scalar.activation(pq, pq, FX.Identity,
                                         bias=bov_t[:, l, dt:dt + 1])
                    nc.vector.tensor_add(
                        x[:, dt, tsl], pq, x[:, dt, tsl])
                # lnf for this half while the other half's wo / ffn runs
                ln_half(lambda d: x[:, d, tsl], lambda d: xf[:, d, tsl],
                        DTILES, DIM)
            if dbg:
                nc.sync.dma_start(out=par[f"d_xatt{l}"][:, :, :], in_=x)
                if l == 0:
                    nc.sync.dma_start(out=par["d_of"][:, :, :], in_=of)

            if l < DEPTH - 1:
                xa = nrm.tile([P, DTILES, TOK], DT_H, name="nrmbuf")
            else:
                xa = nrm.tile([P, DTILES, TOK], DT_H, name="nrmbuf")  # ho
            for th in range(2):
                tsl = bass.ts(th, 512)
                with tc.tile_pool(name=f"h1p{l}_{th}", bufs=1) as h1p:
                    h1r = h1p.tile([P, MTILES, 512], DT_H, name="h1r")
                    for mt in range(MTILES):
                        wt = wsm.tile([P, KTILES, P], DT_H, name="wsm_t")
                        nc.sync.dma_start(out=wt, in_=par["w1t"][l, mt])
                        pq = psm.tile([P, 512], DT_F, name="pmain")
                        for kt in range(KTILES):
                            nc.tensor.matmul(pq, wt[:, kt, :], xf[:, kt, tsl],
                                             start=(kt == 0), stop=(kt == 7))
                        nc.scalar.activation(
                            h1r[:, mt, :], pq, FX.Gelu,
                            bias=b1v_t[:, l, mt:mt + 1], scale=1.0)
                    for dt in range(DTILES):
                        pq = psm.tile([P, 512], DT_F, name="pmain")
                        for kq in range(4):
                            wt = wsm.tile([P, 8, P], DT_H, name="wsm_t")
                            nc.sync.dma_start(
                                out=wt,
                                in_=par["w2t"][l, dt, :,
                                               kq * 8:(kq + 1) * 8, :])
                            for k2 in range(8):
                                kt = kq * 8 + k2
                                nc.tensor.matmul(
                                    pq, wt[:, k2, :], h1r[:, kt, :],
                                    start=(kt == 0), stop=(kt == 31))
                        nc.scalar.activation(pq, pq, FX.Identity,
                                             bias=b2v_t[:, l, dt:dt + 1])
                        nc.vector.tensor_add(
                            x[:, dt, tsl], pq, x[:, dt, tsl])
                # lna of next layer (or final LN) for this half
                ln_half(lambda d: x[:, d, tsl], lambda d: xa[:, d, tsl],
                        DTILES, DIM)
            if dbg:
                nc.sync.dma_start(out=par[f"d_xffn{l}"][:, :, :], in_=x)

        # =================== head (xa == ho, lno folded into wproj) =======
        wt = wsm.tile([P, KTILES, NCLS], DT_H, name="wproj_t")
        nc.sync.dma_start(out=wt, in_=par["wproj"][:, :, :])
        out_sb = const.tile([NCLS, TOK], DT_F, name="out_sb")
        for th in range(2):
            pq = psm.tile([P, 512], DT_F, name="pmain")
            for kt in range(KTILES):
                nc.tensor.matmul(pq[0:NCLS, :], wt[:, kt, :],
                                 xa[:, kt, bass.ts(th, 512)],
                                 start=(kt == 0), stop=(kt == 7))
            nc.scalar.activation(out_sb[:, bass.ts(th, 512)], pq[0:NCLS, :],
                                 FX.Identity, bias=bproj_t)
        nc.sync.dma_start(out=par["out"][:, :], in_=out_sb)


# ============================================================
# host side
# ============================================================

_NC_CACHE = {}


def _bf16(a):
    import ml_dtypes
    return np.ascontiguousarray(a.astype(ml_dtypes.bfloat16))


def _pack_qk(w):      # [D, DIM, 2048] -> [D, 16, P, 8, P]
    v = w.reshape(DEPTH, 8, P, 16, P).transpose(0, 3, 2, 1, 4)
    return _bf16(v)


def _pack_v(w):       # [D, DIM, 1024] -> [D, 2, 8, P, 512]
    v = w.reshape(DEPTH, 8, P, 2, 512).transpose(0, 3, 1, 2, 4)
    return _bf16(v)


def _pack_kxm(w):     # [D, K, M] -> [D, M//P, P, K//P, P]
    D, K, M = w.shape
    v = w.reshape(D, K // P, P, M // P, P).transpose(0, 3, 2, 1, 4)
    return _bf16(v)


def _pack_w2(w):      # [D, 4096, 1024] -> [D, 8, P, 32, P]
    v = w.reshape(DEPTH, 32, P, 8, P).transpose(0, 3, 2, 1, 4)
    return _bf16(v)


def _host_band():
    tt = np.arange(KSIZE, dtype=np.float64)
    kern = np.exp(-0.5 * ((tt - (KSIZE - 1) / 2.0) / SIGMA) ** 2)
    kern = (kern / kern.sum()).astype(np.float32)
    pad_l = (KSIZE - 1) // 2  # 9
    nt = T // P
    bandc = np.zeros((nt, 3, P, P), dtype=np.float32)
    for ct in range(nt):
        for s in range(3):
            kt = ct - 1 + s
            if not (0 <= kt < nt):
                continue
            rows = np.arange(kt * P, (kt + 1) * P)
            cols = np.arange(ct * P, (ct + 1) * P)
            d = rows[:, None] - cols[None, :] + pad_l
            m = (d >= 0) & (d < KSIZE)
            blk = np.zeros((P, P), np.float32)
            blk[m] = kern[d[m]]
            bandc[ct, s] = blk
    return bandc


def _host_etab(rel_tab):
    i = np.arange(SEQ)
    j = i[:, None]
    rel = np.clip(i[None, :] - j, -(MAXREL - 1), MAXREL - 1) + MAXREL - 1
    et = np.zeros((DEPTH, SEQ, SEQ), dtype=np.float32)
    for l in range(DEPTH):
        e = np.exp(rel_tab[l][rel])
        e[j > i[None, :]] = 0.0
        et[l] = e
    return et.reshape(DEPTH, SEQT, P, SEQ)


def kernel(**inputs):
    global _NC_CACHE

    f32 = lambda a: np.ascontiguousarray(np.asarray(a, dtype=np.float32))
    dscale = float(DHEAD) ** -0.5

    wqkv = f32(inputs["Wqkv"])                      # [D, DIM, 3072]
    g_a, b_a = f32(inputs["ln_a_g"]), f32(inputs["ln_a_b"])   # [D, DIM]
    wqk = wqkv[:, :, :2 * DIM] * g_a[:, :, None]
    wqk[:, :, DIM:] *= dscale
    bqk = np.einsum("lk,lkm->lm", b_a, wqkv[:, :, :2 * DIM])
    bqk[:, DIM:] *= dscale
    wv = wqkv[:, :, 2 * DIM:] * g_a[:, :, None]
    bv = np.einsum("lk,lkm->lm", b_a, wqkv[:, :, 2 * DIM:])

    w1 = f32(inputs["W1"])
    g_f, b_f = f32(inputs["ln_f_g"]), f32(inputs["ln_f_b"])
    w1g = w1 * g_f[:, :, None]
    b1f = f32(inputs["b1"]) + np.einsum("lk,lkm->lm", b_f, w1)

    wpe = f32(inputs["W_pe"])
    g_p1, b_p1 = f32(inputs["ln_p1_g"]), f32(inputs["ln_p1_b"])
    wpeg = wpe * g_p1[:, None]
    bpef = f32(inputs["b_pe"]) + b_p1 @ wpe
    # [K,M] -> [M//P, P(k), K//P, P(m)]
    wpe_p = _bf16(wpeg.reshape(KTILES, P, DTILES, P).transpose(2, 1, 0, 3))

    wproj = f32(inputs["Wproj"])
    g_o, b_o = f32(inputs["ln_o_g"]), f32(inputs["ln_o_b"])
    wprojg = wproj * g_o[:, None]
    bprojf = f32(inputs["bproj"]) + b_o @ wproj
    wproj_p = _bf16(wprojg.reshape(KTILES, P, NCLS).transpose(1, 0, 2))

    p2g, p2b = f32(inputs["ln_p2_g"]), f32(inputs["ln_p2_b"])
    p2_trivial = bool(np.all(p2g == 1.0) and np.all(p2b == 0.0))
    v_bias_zero = bool(np.all(bv == 0.0))
    assert v_bias_zero, "nonzero folded v bias not supported in this build"
    qk_bias_zero = bool(np.all(bqk == 0.0))

    key = (p2_trivial, v_bias_zero, qk_bias_zero)
    if key not in _NC_CACHE:
        _NC_CACHE[key] = build_nc(*key)
    nc = _NC_CACHE[key]

    shared = {
        "band": _bf16(_host_band()),
        "etab": _bf16(_host_etab(f32(inputs["rel_tab"]))),
        "wpe": wpe_p,
        "wqk_t": _pack_qk(wqk),
        "wv_t": _pack_v(wv),
        "wot": _pack_kxm(f32(inputs["Wo"])),
        "w1t": _pack_kxm(w1g),
        "w2t": _pack_w2(f32(inputs["W2"])),
        "wproj": wproj_p,
        "bpe": bpef,
        "lnp2g": p2g, "lnp2b": p2b,
        "bqk": np.ascontiguousarray(bqk, dtype=np.float32),
        "bv": np.ascontiguousarray(bv, dtype=np.float32),
        "bov": f32(inputs["bo"]), "b1v": np.ascontiguousarray(b1f, np.float32),
        "b2v": f32(inputs["b2"]),
        "bprojv": np.ascontiguousarray(bprojf, np.float32),
    }
    xfull = f32(inputs["neuralInput"])
    in_maps = []
    for c in range(NCORES):
        m = dict(shared)
        m["xin"] = _bf16(xfull[c * BPC:(c + 1) * BPC])
        in_maps.append(m)

    import os
    trace = bool(os.environ.get("BIT_TRACE"))
    res = run_bass_kernel_spmd(nc, in_maps, list(range(NCORES)), trace=trace)
    if trace:
        globals()["LAST_RESULT"] = res
    outs = []
    for c in range(NCORES):
        o = res.results[c]["out"]              # [NCLS, TOK]
        o = o.reshape(NCLS, BPC, SEQ).transpose(1, 2, 0)
        outs.append(o)
    return np.concatenate(outs, axis=0).astype(np.float32)


# revision 26
# speedup vs baseline: 1.0255x; 1.0047x over previous
"""Trainium2 Bass kernel for nn_BiT_Phoneme (dense transformer).

Data-parallel: 16 batch elems / 8 cores = 2 per core; each core runs the
full network on its 2 sequences (1024 "tokens"). Activations are kept
feature-major ([dim on partitions, tokens on free]) so matmuls chain
without transposes. The gaussian time-smoothing + patchify are a banded
matmul with a host-precomputed band matrix.

v2 redesign (vs baseline at 4.30 ms):
- LayerNorm gains/biases are folded into the *following* weight matrix on
  the host (W' = g*W, bias' += b@W), so the on-device LN apply is just
  (x - mu)*rstd -> bf16: 2 DVE ops/tile instead of 3, and the affine
  params never touch the device.
- rstd = reciprocal_approx_fast(sqrt(var+eps)) instead of the exact
  6 us/call nc.vector.reciprocal.
- mu/rstd partition-broadcasts run on GpSimd (partition_broadcast), not
  the PE, so LN never blocks the in-order PE queue.
- LNs are computed per 512-token half, and emission is dataflow-ordered:
  each half's stats+apply are emitted immediately after the phase that
  produces that half (wo b0 -> lnf h0 -> wo b1 -> lnf h1 -> ffn th0 ->
  lna' h0 -> ffn th1 -> lna' h1 -> next layer). The PE stays busy with
  the other half's matmuls while DVE/Scalar/GpSimd do the LN.
- Softmax denominators come free from a 65th all-ones column appended to
  each head's V block (probs@V matmul row 64 = sum of probs), removing
  768 M=1 PE matmuls.
- Score matmuls (K=64) for head pairs are emitted adjacently with
  explicit tile_position (0,0)/(64,0) so the two 64-row matmuls run
  concurrently in the PE array.
- q/k/v/wo/ffn drains moved to the Scalar engine (activation with
  per-partition bias) to balance DVE load.
Matmul dtypes: weights/activations bf16 (x residual stream fp32r).
"""

import numpy as np

import concourse.bass as bass
import concourse.mybir as mybir
import concourse.tile as tile
from concourse import bacc
from concourse.bass_utils import run_bass_kernel_spmd

B, T, F = 16, 2048, 256
PH = 4
PATCH = 1024
DIM = 1024
DEPTH = 6
HEADS, DHEAD = 16, 64
INNER = 1024
MLP = 4096
NCLS = 41
MAXREL = 200
KSIZE, SIGMA = 20, 2.0
EPS = 1e-5
SEQ = T // PH              # 512
NCORES = 8
BPC = B // NCORES          # 2
TOK = BPC * SEQ            # 1024
P = 128

DT_R = mybir.dt.float32r
DT_F = mybir.dt.float32
DT_H = mybir.dt.bfloat16
FX = mybir.ActivationFunctionType
OP = mybir.AluOpType

DTILES = DIM // P          # 8
KTILES = DIM // P          # 8
MTILES = MLP // P          # 32
SEQT = SEQ // P            # 4


def build_nc(p2_trivial, v_bias_zero, qk_bias_zero=True, dbg=False):
    nc = bacc.Bacc(None, target_bir_lowering=False)

    par = {}
    def dp(name, shape, dtype, is_out=False):
        par[name] = nc.declare_dram_parameter(name, list(shape), dtype, isOutput=is_out)
        return par[name]

    dp("xin", (BPC, T, F), DT_H)
    dp("band", (T // P, 3, P, P), DT_H)
    dp("etab", (DEPTH, SEQT, P, SEQ), DT_H)
    dp("wpe", (DTILES, P, KTILES, P), DT_H)
    dp("wqk_t", (DEPTH, 16, P, KTILES, P), DT_H)
    dp("wv_t", (DEPTH, 2, KTILES, P, 512), DT_H)
    dp("wot", (DEPTH, DTILES, P, KTILES, P), DT_H)
    dp("w1t", (DEPTH, MTILES, P, KTILES, P), DT_H)
    dp("w2t", (DEPTH, DTILES, P, 32, P), DT_H)
    dp("wproj", (P, KTILES, NCLS), DT_H)
    for nm, shp in [("bpe", (DIM,)),
                    ("lnp2g", (DIM,)), ("lnp2b", (DIM,)),
                    ("bqk", (DEPTH, 2 * DIM)), ("bv", (DEPTH, INNER)),
                    ("bov", (DEPTH, DIM)), ("b1v", (DEPTH, MLP)),
                    ("b2v", (DEPTH, DIM)), ("bprojv", (NCLS,))]:
        dp(nm, shp, DT_F)
    dp("out", (NCLS, TOK), DT_F, is_out=True)
    if dbg:
        dp("d_xemb", (P, DTILES, TOK), DT_R, is_out=True)
        dp("d_xa0", (P, DTILES, TOK), DT_H, is_out=True)
        dp("d_qf", (P, DTILES, 512), DT_H, is_out=True)
        dp("d_kf", (P, DTILES, 512), DT_H, is_out=True)
        dp("d_vt", (P, SEQT, HEADS, DHEAD), DT_H, is_out=True)
        dp("d_etr", (P, SEQT, 512), DT_H, is_out=True)
        dp("d_of", (P, DTILES, TOK), DT_H, is_out=True)
        for l in range(DEPTH):
            dp(f"d_xatt{l}", (P, DTILES, TOK), DT_R, is_out=True)
            dp(f"d_xffn{l}", (P, DTILES, TOK), DT_R, is_out=True)

    with tile.TileContext(nc) as tc:
        _emit(nc, tc, par, p2_trivial, v_bias_zero, qk_bias_zero, dbg)
    nc.compile()
    return nc


def _emit(nc, tc, par, p2_trivial, v_bias_zero, qk_bias_zero=True, dbg=False):
    import contextlib
    ctx = contextlib.ExitStack()
    with ctx:
        const = ctx.enter_context(tc.tile_pool(name="const", bufs=1))
        xpool = ctx.enter_context(tc.tile_pool(name="xpool", bufs=1))
        nrm = ctx.enter_context(tc.tile_pool(name="nrm", bufs=2))
        wsm = ctx.enter_context(tc.tile_pool(name="wsm", bufs=4))
        stats = ctx.enter_context(tc.tile_pool(name="stats", bufs=2))
        lnb = ctx.enter_context(tc.tile_pool(name="lnb", bufs=2))
        sm2 = ctx.enter_context(tc.tile_pool(name="sm2", bufs=4))
        atp = ctx.enter_context(tc.tile_pool(name="atp", bufs=2))
        etp = ctx.enter_context(tc.tile_pool(name="etp", bufs=1))
        psm = ctx.enter_context(tc.tile_pool(name="psm", bufs=3, space="PSUM"))
        pso = ctx.enter_context(tc.tile_pool(name="pso", bufs=3, space="PSUM"))
        pst = ctx.enter_context(tc.tile_pool(name="pst", bufs=1, space="PSUM"))

        ones_r = const.tile([P, 1], DT_R, name="ones_r")
        nc.vector.memset(ones_r.bitcast(mybir.dt.uint32), 0x3F800000)
        epst = const.tile([1, 1], DT_F, name="epst")
        nc.vector.memset(epst, EPS)
        ones_h = const.tile([P, 1], DT_H, name="ones_h")
        nc.vector.memset(ones_h.bitcast(mybir.dt.uint16), 0x3F80)

        def load_vec(nm, width):
            d = par[nm]
            if len(d.shape) == 1:
                tl = const.tile([P, width // P], DT_F, name=nm + "_t")
                nc.sync.dma_start(out=tl, in_=d.rearrange("(o p) -> p o", p=P))
            else:
                L = d.shape[0]
                tl = const.tile([P, L, width // P], DT_F, name=nm + "_t")
                nc.sync.dma_start(out=tl, in_=d.rearrange("l (o p) -> p l o", p=P))
            return tl

        bpe_t = load_vec("bpe", DIM)
        bqk_t = load_vec("bqk", 2 * DIM)
        bov_t = load_vec("bov", DIM)
        b1v_t = load_vec("b1v", MLP)
        b2v_t = load_vec("b2v", DIM)
        if not p2_trivial:
            lnp2g_t = load_vec("lnp2g", DIM)
            lnp2b_t = load_vec("lnp2b", DIM)
        bproj_t = const.tile([NCLS, 1], DT_F, name="bproj_t")
        nc.sync.dma_start(out=bproj_t,
                          in_=par["bprojv"].rearrange("(p o) -> p o", o=1))

        x = xpool.tile([P, DTILES, TOK], DT_R, name="x")

        # ---- one 512-wide LN half: stats -> mu/rstd -> bcast -> apply ----
        # views(d): [P,512] fp32(r) source; dst(d): [P,512] output slice.
        # gb: optional (g_fn, b_fn) per-partition affine (non-folded path).
        def ln_stats_alloc():
            ps0 = pst.tile([1, 512], DT_F, name="ps0")
            ps1 = pst.tile([1, 512], DT_F, name="ps1")
            return ps0, ps1

        def ln_stats_tile(ps, view, d, ntiles, hsrc=False):
            ps0, ps1 = ps
            onev = ones_h if hsrc else ones_r
            sq = sm2.tile([P, 512], DT_H if hsrc else DT_R, name="sq")
            nc.scalar.square(sq, view)
            nc.tensor.matmul(ps0, onev, view,
                             start=(d == 0), stop=(d == ntiles - 1))
            nc.tensor.matmul(ps1, onev, sq,
                             start=(d == 0), stop=(d == ntiles - 1))

        def ln_finish(ps, views, dst, ntiles, D, gb=None):
            ps0, ps1 = ps
            rows = stats.tile([1, 3, 512], DT_F, name="rows")
            mu, var, rstd = rows[:, 0, :], rows[:, 1, :], rows[:, 2, :]
            nc.vector.tensor_scalar(mu, ps0, 1.0 / D, None, OP.mult)
            nc.vector.tensor_scalar(var, ps1, 1.0 / D, None, OP.mult)
            nc.vector.tensor_mul(rstd, mu, mu)
            nc.vector.tensor_sub(var, var, rstd)
            nc.scalar.activation(var, var, FX.Sqrt, bias=epst, scale=1.0)
            with nc.allow_low_precision(reason="approx rstd for LN"):
                nc.vector.reciprocal_approx_fast(out=rstd, in_=var)
            mub = lnb.tile([P, 512], DT_F, name="mub")
            rsb = lnb.tile([P, 512], DT_F, name="rsb")
            nc.gpsimd.partition_broadcast(mub, mu)
            nc.gpsimd.partition_broadcast(rsb, rstd)
            for d in range(ntiles):
                t = sm2.tile([P, 512], DT_F, name="lnt")
                nc.vector.tensor_sub(t, views(d), mub)
                if gb is None:
                    nc.vector.tensor_mul(dst(d), t, rsb)
                else:
                    nc.vector.tensor_mul(t, t, rsb)
                    nc.vector.tensor_scalar(dst(d), t, gb[0](d), gb[1](d),
                                            OP.mult, OP.add)

        def ln_half(views, dst, ntiles, D, gb=None, hsrc=False):
            ps = ln_stats_alloc()
            for d in range(ntiles):
                ln_stats_tile(ps, views(d), d, ntiles, hsrc=hsrc)
            ln_finish(ps, views, dst, ntiles, D, gb=gb)

        # =================== embedding ===================
        xin, band = par["xin"], par["band"]
        xa = nrm.tile([P, DTILES, TOK], DT_H, name="nrmbuf")   # lna(0) out
        with (
            tc.tile_pool(name="sfp", bufs=1) as sfp,
            tc.tile_pool(name="pnp", bufs=1) as pnp,
            tc.tile_pool(name="xtp", bufs=2) as xtp,
            tc.tile_pool(name="bnp", bufs=1) as bnp,
        ):
            bandt = bnp.tile([P, T // P, 3, P], DT_H, name="bandt")
            nc.sync.dma_start(out=bandt,
                              in_=band.rearrange("c s p q -> p c s q"))
            sfs, pns, xts = [], [], []

            def emit_xt(b):
                xt = xtp.tile([P, T // P, F], DT_H, name="xt")
                nc.sync.dma_start(
                    out=xt, in_=xin[b].rearrange("(kt p) f -> p kt f", p=P))
                xts.append(xt)

            def emit_band(b):
                xt = xts[b]
                sf = sfp.tile([P, 2, T], DT_H, name="sf")
                sfs.append(sf)
                for fh in range(2):
                    for g4 in range(T // 512):
                        pg = psm.tile([P, 512], DT_F, name="pmain")
                        for q in range(4):
                            ct = g4 * 4 + q
                            svals = [s for s in range(3)
                                     if 0 <= ct - 1 + s < T // P]
                            for si, s in enumerate(svals):
                                kt = ct - 1 + s
                                nc.tensor.matmul(
                                    pg[:, bass.ts(q, P)],
                                    xt[:, kt, bass.ts(fh, P)],
                                    bandt[:, ct, s, :],
                                    start=(si == 0),
                                    stop=(si == len(svals) - 1))
                        nc.vector.tensor_copy(sf[:, fh, bass.ts(g4, 512)], pg)

            def emit_lnp1(b):
                sf = sfs[b]

                def pview(pt):
                    i, fh = pt // 2, pt % 2
                    return sf[:, fh, :].rearrange(
                        "p (s four) -> p four s", four=PH)[:, i, :]

                pn = pnp.tile([P, 8, 512], DT_H, name="pn")
                pns.append(pn)
                ln_half(pview, lambda d: pn[:, d, :], 8, PATCH, hsrc=True)

            def emit_pe(b):
                pn = pns[b]
                for dt in range(DTILES):
                    wt = wsm.tile([P, KTILES, P], DT_H, name="wsm_t")
                    nc.sync.dma_start(out=wt, in_=par["wpe"][dt])
                    pq = psm.tile([P, 512], DT_F, name="pmain")
                    for kt in range(KTILES):
                        nc.tensor.matmul(pq, wt[:, kt, :], pn[:, kt, :],
                                         start=(kt == 0), stop=(kt == 7))
                    nc.scalar.activation(x[:, dt, bass.ts(b, 512)], pq,
                                         FX.Identity, bias=bpe_t[:, dt:dt + 1])

            def emit_lnp2(b):
                # in-place: x <- (x - mu) * rstd (stats read x pre-apply)
                sl = bass.ts(b, 512)
                gb = None
                if not p2_trivial:
                    gb = (lambda d: lnp2g_t[:, d:d + 1],
                          lambda d: lnp2b_t[:, d:d + 1])
                ln_half(lambda d: x[:, d, sl], lambda d: x[:, d, sl],
                        DTILES, DIM, gb=gb)

            def emit_lna0(b):
                sl = bass.ts(b, 512)
                ln_half(lambda d: x[:, d, sl], lambda d: xa[:, d, sl],
                        DTILES, DIM)

            emit_xt(0)
            emit_xt(1)
            emit_band(0)
            emit_lnp1(0)
            emit_pe(0)
            emit_band(1)
            emit_lnp2(0)
            emit_lnp1(1)
            emit_pe(1)
            emit_lna0(0)
            emit_lnp2(1)
            emit_lna0(1)
        if dbg:
            nc.sync.dma_start(out=par["d_xemb"][:, :, :], in_=x)
            nc.sync.dma_start(out=par["d_xa0"][:, :, :], in_=xa)

        # =================== transformer layers ===================
        for l in range(DEPTH):
            et_sb = etp.tile([P, SEQT, SEQ], DT_H, name="et_sb")
            nc.sync.dma_start(out=et_sb,
                              in_=par["etab"][l].rearrange("jt p i -> p jt i"))

            of = nrm.tile([P, DTILES, TOK], DT_H, name="nrmbuf")
            for b in range(BPC):
                tsl = bass.ts(b, 512)
                with tc.tile_pool(name=f"qkv{l}_{b}", bufs=1) as qkvp:
                    qf = qkvp.tile([P, DTILES, 512], DT_H, name="qf")
                    kf = qkvp.tile([P, DTILES, 512], DT_H, name="kf")
                    vt = qkvp.tile([P, SEQT, HEADS, DHEAD + 1], DT_H,
                                   name="vt")
                    nc.vector.memset(vt[:, :, :, DHEAD], 1.0)
                    # q, k feature-major (g_a and 1/sqrt(dhead) folded into W)
                    for c in range(2 * DTILES):
                        wt = wsm.tile([P, KTILES, P], DT_H, name="wsm_t")
                        nc.sync.dma_start(out=wt, in_=par["wqk_t"][l, c])
                        pq = psm.tile([P, 512], DT_F, name="pmain")
                        for kt in range(KTILES):
                            nc.tensor.matmul(pq, wt[:, kt, :], xa[:, kt, tsl],
                                             start=(kt == 0), stop=(kt == 7))
                        dst = qf if c < DTILES else kf
                        nc.scalar.activation(dst[:, c % DTILES, :], pq,
                                             FX.Identity,
                                             bias=bqk_t[:, l, c:c + 1])
                    # v token-major with ones column per head
                    for nh in range(2):
                        for tth in range(2):
                            tts = (2 * tth, 2 * tth + 1)
                            pvs = [psm.tile([P, 512], DT_F, name="pmain")
                                   for _ in tts]
                            for kt in range(KTILES):
                                wv = wsm.tile([P, 512], DT_H, name="wsm_t")
                                nc.sync.dma_start(
                                    out=wv, in_=par["wv_t"][l, nh, kt])
                                for ti, tt in enumerate(tts):
                                    nc.tensor.matmul(
                                        pvs[ti],
                                        xa[:, kt, b * 512 + tt * P:
                                           b * 512 + (tt + 1) * P],
                                        wv,
                                        start=(kt == 0), stop=(kt == 7))
                            for ti, tt in enumerate(tts):
                                nc.scalar.activation(
                                    vt[:, tt, nh * 8:(nh + 1) * 8, 0:DHEAD],
                                    pvs[ti].rearrange("p (h d) -> p h d",
                                                      h=8),
                                    FX.Identity)

                    for dt in range(DTILES):
                        hA, hB = 2 * dt, 2 * dt + 1
                        etrA = atp.tile([P, SEQT, 512], DT_H, name="etr")
                        etrB = atp.tile([P, SEQT, 512], DT_H, name="etr")
                        if dbg and l == 0 and b == 0 and dt == 0:
                            nc.vector.memset(etrA, 0.0)
                        for jt in range(SEQT):
                            i0 = jt * P
                            jts = bass.ts(jt, P)
                            scA = psm.tile([P, 512], DT_F, name="pmain")
                            scB = psm.tile([P, 512], DT_F, name="pmain")
                            nc.tensor.matmul(
                                scA[:, i0:], kf[0:64, dt, jts],
                                qf[0:64, dt, i0:],
                                start=True, stop=True, tile_position=(0, 0))
                            nc.tensor.matmul(
                                scB[:, i0:], kf[64:128, dt, jts],
                                qf[64:128, dt, i0:],
                                start=True, stop=True, tile_position=(64, 0))
                            for sc, etr in ((scA, etrA), (scB, etrB)):
                                ex = sm2.tile([P, 512], DT_F, name="ex")
                                nc.scalar.activation(ex[:, i0:], sc[:, i0:],
                                                     FX.Exp)
                                nc.vector.tensor_mul(
                                    etr[:, jt, i0:], ex[:, i0:],
                                    et_sb[:, jt, i0:])
                        if dbg and l == 0 and b == 0 and dt == 0:
                            nc.sync.dma_start(out=par["d_etr"][:, :, :],
                                              in_=etrA)
                        for hd, etr in ((hA, etrA), (hB, etrB)):
                            ot = pso.tile([P, 512], DT_F, name="ot")
                            for jt in range(SEQT):
                                i0 = jt * P
                                nc.tensor.matmul(
                                    ot[0:DHEAD + 1, i0:],
                                    vt[:, jt, hd, :],
                                    etr[:, jt, i0:],
                                    start=(jt == 0), stop=(jt == SEQT - 1))
                            # den sits at PSUM row 64; DVE can read it in
                            # place (no partition shift), the SBUF->SBUF DMA
                            # moves it to partition 0 for recip+broadcast.
                            dsb = stats.tile([65, 512], DT_F, name="dsb")
                            nc.vector.tensor_copy(dsb[64:65, :],
                                                  ot[64:65, :])
                            den0 = stats.tile([1, 512], DT_F, name="den0")
                            nc.sync.dma_start(out=den0, in_=dsb[64:65, :])
                            adr1 = stats.tile([1, 512], DT_F, name="adr1")
                            nc.vector.reciprocal_approx_fast(
                                out=adr1, in_=den0)
                            adrb = stats.tile([64, 512], DT_F, name="adrb")
                            nc.gpsimd.partition_broadcast(adrb, adr1)
                            if hd % 2 == 0:
                                nc.vector.tensor_mul(
                                    of[0:64, dt, tsl], ot[0:64, :], adrb)
                            else:
                                # DVE can't shift partitions: normalize at
                                # base 0, DMA-copy up to partitions 64:128.
                                otmp = sm2.tile([64, 512], DT_H, name="otmp")
                                nc.vector.tensor_mul(otmp, ot[0:64, :], adrb)
                                nc.sync.dma_start(
                                    out=of[64:128, dt, tsl], in_=otmp)
                    if dbg and l == 0 and b == 0:
                        nc.sync.dma_start(out=par["d_qf"][:, :, :], in_=qf)
                        nc.sync.dma_start(out=par["d_kf"][:, :, :], in_=kf)
                        nc.sync.dma_start(out=par["d_vt"][:, :, :, :],
                                          in_=vt)

            xf = nrm.tile([P, DTILES, TOK], DT_H, name="nrmbuf")
            for b in range(BPC):
                tsl = bass.ts(b, 512)
                for dt in range(DTILES):
                    wt = wsm.tile([P, KTILES, P], DT_H, name="wsm_t")
                    nc.sync.dma_start(out=wt, in_=par["wot"][l, dt])
                    pq = psm.tile([P, 512], DT_F, name="pmain")
                    for kt in range(KTILES):
                        nc.tensor.matmul(
                            pq, wt[:, kt, :], of[:, kt, tsl],
                            start=(kt == 0), stop=(kt == 7))
                    nc.scalar.activation(pq, pq, FX.Identity,
                                         bias=bov_t[:, l, dt:dt + 1])
                    nc.vector.tensor_add(
                        x[:, dt, tsl], pq, x[:, dt, tsl])
                # lnf for this half while the other half's wo / ffn runs
                ln_half(lambda d: x[:, d, tsl], lambda d: xf[:, d, tsl],
                        DTILES, DIM)
            if dbg:
                nc.sync.dma_start(out=par[f"d_xatt{l}"][:, :, :], in_=x)
                if l == 0:
                    nc.sync.dma_start(out=par["d_of"][:, :, :], in_=of)

            if l < DEPTH - 1:
                xa = nrm.tile([P, DTILES, TOK], DT_H, name="nrmbuf")
            else:
                xa = nrm.tile([P, DTILES, TOK], DT_H, name="nrmbuf")  # ho
            for th in range(2):
                tsl = bass.ts(th, 512)
                with tc.tile_pool(name=f"h1p{l}_{th}", bufs=1) as h1p:
                    h1r = h1p.tile([P, MTILES, 512], DT_H, name="h1r")
                    for mt in range(MTILES):
                        wt = wsm.tile([P, KTILES, P], DT_H, name="wsm_t")
                        nc.sync.dma_start(out=wt, in_=par["w1t"][l, mt])
                        pq = psm.tile([P, 512], DT_F, name="pmain")
                        for kt in range(KTILES):
                            nc.tensor.matmul(pq, wt[:, kt, :], xf[:, kt, tsl],
                                             start=(kt == 0), stop=(kt == 7))
                        nc.scalar.activation(
                            h1r[:, mt, :], pq, FX.Gelu,
                            bias=b1v_t[:, l, mt:mt + 1], scale=1.0)
                    for dt in range(DTILES):
                        pq = psm.tile([P, 512], DT_F, name="pmain")
                        for kq in range(4):
                            wt = wsm.tile([P, 8, P], DT_H, name="wsm_t")
                            nc.sync.dma_start(
                                out=wt,
                                in_=par["w2t"][l, dt, :,
                                               kq * 8:(kq + 1) * 8, :])
                            for k2 in range(8):
                                kt = kq * 8 + k2
                                nc.tensor.matmul(
                                    pq, wt[:, k2, :], h1r[:, kt, :],
                                    start=(kt == 0), stop=(kt == 31))
                        nc.scalar.activation(pq, pq, FX.Identity,
                                             bias=b2v_t[:, l, dt:dt + 1])
                        nc.vector.tensor_add(
                            x[:, dt, tsl], pq, x[:, dt, tsl])
                # lna of next layer (or final LN) for this half
                ln_half(lambda d: x[:, d, tsl], lambda d: xa[:, d, tsl],
                        DTILES, DIM)
            if dbg:
                nc.sync.dma_start(out=par[f"d_xffn{l}"][:, :, :], in_=x)

        # =================== head (xa == ho, lno folded into wproj) =======
        wt = wsm.tile([P, KTILES, NCLS], DT_H, name="wproj_t")
        nc.sync.dma_start(out=wt, in_=par["wproj"][:, :, :])
        out_sb = const.tile([NCLS, TOK], DT_F, name="out_sb")
        for th in range(2):
            pq = psm.tile([P, 512], DT_F, name="pmain")
            for kt in range(KTILES):
                nc.tensor.matmul(pq[0:NCLS, :], wt[:, kt, :],
                                 xa[:, kt, bass.ts(th, 512)],
                                 start=(kt == 0), stop=(kt == 7))
            nc.scalar.activation(out_sb[:, bass.ts(th, 512)], pq[0:NCLS, :],
                                 FX.Identity, bias=bproj_t)
        nc.sync.dma_start(out=par["out"][:, :], in_=out_sb)


# ============================================================
# host side
# ============================================================

_NC_CACHE = {}


def _bf16(a):
    import ml_dtypes
    return np.ascontiguousarray(a.astype(ml_dtypes.bfloat16))


def _pack_qk(w):      # [D, DIM, 2048] -> [D, 16, P, 8, P]
    v = w.reshape(DEPTH, 8, P, 16, P).transpose(0, 3, 2, 1, 4)
    return _bf16(v)


def _pack_v(w):       # [D, DIM, 1024] -> [D, 2, 8, P, 512]
    v = w.reshape(DEPTH, 8, P, 2, 512).transpose(0, 3, 1, 2, 4)
    return _bf16(v)


def _pack_kxm(w):     # [D, K, M] -> [D, M//P, P, K//P, P]
    D, K, M = w.shape
    v = w.reshape(D, K // P, P, M // P, P).transpose(0, 3, 2, 1, 4)
    return _bf16(v)


def _pack_w2(w):      # [D, 4096, 1024] -> [D, 8, P, 32, P]
    v = w.reshape(DEPTH, 32, P, 8, P).transpose(0, 3, 2, 1, 4)
    return _bf16(v)


def _host_band():
    tt = np.arange(KSIZE, dtype=np.float64)
    kern = np.exp(-0.5 * ((tt - (KSIZE - 1) / 2.0) / SIGMA) ** 2)
    kern = (kern / kern.sum()).astype(np.float32)
    pad_l = (KSIZE - 1) // 2  # 9
    nt = T // P
    bandc = np.zeros((nt, 3, P, P), dtype=np.float32)
    for ct in range(nt):
        for s in range(3):
            kt = ct - 1 + s
            if not (0 <= kt < nt):
                continue
            rows = np.arange(kt * P, (kt + 1) * P)
            cols = np.arange(ct * P, (ct + 1) * P)
            d = rows[:, None] - cols[None, :] + pad_l
            m = (d >= 0) & (d < KSIZE)
            blk = np.zeros((P, P), np.float32)
            blk[m] = kern[d[m]]
            bandc[ct, s] = blk
    return bandc


def _host_etab(rel_tab):
    i = np.arange(SEQ)
    j = i[:, None]
    rel = np.clip(i[None, :] - j, -(MAXREL - 1), MAXREL - 1) + MAXREL - 1
    et = np.zeros((DEPTH, SEQ, SEQ), dtype=np.float32)
    for l in range(DEPTH):
        e = np.exp(rel_tab[l][rel])
        e[j > i[None, :]] = 0.0
        et[l] = e
    return et.reshape(DEPTH, SEQT, P, SEQ)


def kernel(**inputs):
    global _NC_CACHE

    f32 = lambda a: np.ascontiguousarray(np.asarray(a, dtype=np.float32))
    dscale = float(DHEAD) ** -0.5

    wqkv = f32(inputs["Wqkv"])                      # [D, DIM, 3072]
    g_a, b_a = f32(inputs["ln_a_g"]), f32(inputs["ln_a_b"])   # [D, DIM]
    wqk = wqkv[:, :, :2 * DIM] * g_a[:, :, None]
    wqk[:, :, DIM:] *= dscale
    bqk = np.einsum("lk,lkm->lm", b_a, wqkv[:, :, :2 * DIM])
    bqk[:, DIM:] *= dscale
    wv = wqkv[:, :, 2 * DIM:] * g_a[:, :, None]
    bv = np.einsum("lk,lkm->lm", b_a, wqkv[:, :, 2 * DIM:])

    w1 = f32(inputs["W1"])
    g_f, b_f = f32(inputs["ln_f_g"]), f32(inputs["ln_f_b"])
    w1g = w1 * g_f[:, :, None]
    b1f = f32(inputs["b1"]) + np.einsum("lk,lkm->lm", b_f, w1)

    wpe = f32(inputs["W_pe"])
    g_p1, b_p1 = f32(inputs["ln_p1_g"]), f32(inputs["ln_p1_b"])
    wpeg = wpe * g_p1[:, None]
    bpef = f32(inputs["b_pe"]) + b_p1 @ wpe
    # [K,M] -> [M//P, P(k), K//P, P(m)]
    wpe_p = _bf16(wpeg.reshape(KTILES, P, DTILES, P).transpose(2, 1, 0, 3))

    wproj = f32(inputs["Wproj"])
    g_o, b_o = f32(inputs["ln_o_g"]), f32(inputs["ln_o_b"])
    wprojg = wproj * g_o[:, None]
    bprojf = f32(inputs["bproj"]) + b_o @ wproj
    wproj_p = _bf16(wprojg.reshape(KTILES, P, NCLS).transpose(1, 0, 2))

    p2g, p2b = f32(inputs["ln_p2_g"]), f32(inputs["ln_p2_b"])
    p2_trivial = bool(np.all(p2g == 1.0) and np.all(p2b == 0.0))
    v_bias_zero = bool(np.all(bv == 0.0))
    assert v_bias_zero, "nonzero folded v bias not supported in this build"
    qk_bias_zero = bool(np.all(bqk == 0.0))

    key = (p2_trivial, v_bias_zero, qk_bias_zero)
    if key not in _NC_CACHE:
        _NC_CACHE[key] = build_nc(*key)
    nc = _NC_CACHE[key]

    shared = {
        "band": _bf16(_host_band()),
        "etab": _bf16(_host_etab(f32(inputs["rel_tab"]))),
        "wpe": wpe_p,
        "wqk_t": _pack_qk(wqk),
        "wv_t": _pack_v(wv),
        "wot": _pack_kxm(f32(inputs["Wo"])),
        "w1t": _pack_kxm(w1g),
        "w2t": _pack_w2(f32(inputs["W2"])),
        "wproj": wproj_p,
        "bpe": bpef,
        "lnp2g": p2g, "lnp2b": p2b,
        "bqk": np.ascontiguousarray(bqk, dtype=np.float32),
        "bv": np.ascontiguousarray(bv, dtype=np.float32),
        "bov": f32(inputs["bo"]), "b1v": np.ascontiguousarray(b1f, np.float32),
        "b2v": f32(inputs["b2"]),
        "bprojv": np.ascontiguousarray(bprojf, np.float32),
    }
    xfull = f32(inputs["neuralInput"])
    in_maps = []
    for c in range(NCORES):
        m = dict(shared)
        m["xin"] = _bf16(xfull[c * BPC:(c + 1) * BPC])
        in_maps.append(m)

    import os
    trace = bool(os.environ.get("BIT_TRACE"))
    res = run_bass_kernel_spmd(nc, in_maps, list(range(NCORES)), trace=trace)
    if trace:
        globals()["LAST_RESULT"] = res
    outs = []
    for c in range(NCORES):
        o = res.results[c]["out"]              # [NCLS, TOK]
        o = o.reshape(NCLS, BPC, SEQ).transpose(1, 2, 0)
        outs.append(o)
    return np.concatenate(outs, axis=0).astype(np.float32)
